# revision 1
# baseline (speedup 1.0000x reference)
"""ChildSum TreeLSTM (N=8192 nodes, 4-ary static heap tree, H=256, D=300) on 8 trn2 NeuronCores.

Strategy
--------
The tree is static: node i's children are 4i+1..4i+4 (clipped at N). The reverse
scan (children before parents) is equivalent to processing the tree level by
level, bottom-up; nodes within a level are independent, so each level is a
batched LSTM cell (matmuls + elementwise).

Sharding: the 256 level-4 subtrees are partitioned across the 8 cores (balanced
by the number of *internal* level-6 descendants, which determines level-7 leaf
count). Each core processes its forest fully locally — children of a sorted node
range are contiguous in the next level's sorted array, so the recurrence needs
no gathers and no cross-core communication. Cores output their 32 level-4 root
(h, c) states; the tiny top of the tree (levels 3..0, 85 nodes) plus the final
log_softmax run on the host in numpy.

On-device layout: everything is transposed — feature dim on SBUF partitions
(256 features = 2 halves of 128), nodes along the free axis. The child-h sums
and per-child forget gates then become strided slicing along the free axis, and
the x-side / h-side gate projections accumulate into the same PSUM tile.
Biases (bx + bh, zeros in practice) are folded into an extra ones-row of the
x-side matmul, so pad columns (zero x) self-compute to h = c = 0.
"""

import numpy as np
import ml_dtypes

BF16 = ml_dtypes.bfloat16

N = 8192
H = 256
D = 300
K = 4
OUT = 4
NCORES = 8
L7P = 384           # padded level-7 columns per core (4 * IPMAX)
IPMAX = 96          # max internal level-6 nodes per core
KDIM = 304          # padded contraction rows of xt/wx (300 emb + 1 ones + pad)
KUSE = 301          # rows actually used in matmuls
XCOLS = L7P + 512 + 128 + 32   # 1056 per-core node columns: [L7 | L6 | L5 | L4]

GATE_MAP = [0, 2, 3, 1]  # our gate order [i, o, u, f] -> reference gate indices

F32 = np.float32


def _build_plan():
    """Assign the 256 level-4 subtrees to 8 cores; build per-core column maps."""
    # w(u) = number of internal (has-children) level-6 descendants of L4 node u.
    # Full-weight subtrees (w=16) are u in [85, 127); u=127 has w=11; rest 0.
    full = list(range(85, 127))                               # 42 subtrees
    lights = list(range(128, 341))                            # 213 subtrees
    heavy_counts = [6, 6, 5, 5, 5, 5, 5, 5]                   # sums to 42
    light_counts = [26, 26, 26, 27, 27, 27, 27, 27]           # sums to 213
    cores = []
    hpos = 0
    lpos = 0
    for c in range(NCORES):
        hs = full[hpos:hpos + heavy_counts[c]]
        hpos += heavy_counts[c]
        if c == 2:
            hs = hs + [127]                                   # w sums: 96,96,91,80*5
        ls = lights[lpos:lpos + light_counts[c]]
        lpos += light_counts[c]
        cores.append(sorted(hs + ls))
    all_l4 = sorted(u for cs in cores for u in cs)
    assert all_l4 == list(range(85, 341)), "L4 assignment must partition [85, 341)"

    plan = []
    for c in range(NCORES):
        l4 = cores[c]
        assert len(l4) == 32
        l5 = [4 * u + 1 + k for u in l4 for k in range(K)]
        l6 = [4 * v + 1 + k for v in l5 for k in range(K)]
        wc = sum(1 for x in l6 if x < 2048)
        assert wc <= IPMAX
        l7 = []
        for x in l6[:wc]:
            for k in range(K):
                ch = 4 * x + 1 + k
                l7.append(ch if ch < N else -1)
        l7 += [-1] * (L7P - len(l7))
        cols = np.array(l7 + l6 + l5 + l4, dtype=np.int64)
        assert cols.shape == (XCOLS,)
        plan.append((cols, wc, np.array(l4, dtype=np.int64)))
    return plan


_PLAN = _build_plan()

# chunk schedule: (xoff, ncols, ip, child_level, child_col_off, out_level, out_off)
# child/out levels refer to state buffers keyed 7, 6, 5, 4. L5 is split in two
# so its halves chain off the two independent L6 chunks — the scheduler can
# overlap one chain's elementwise tail with the other chain's matmuls.
_CHUNKS = [
    (0,    256, 0,   None, 0,   7, 0),     # L7 leaves, part A
    (256,  128, 0,   None, 0,   7, 256),   # L7 leaves, part B
    (640,  256, 0,   None, 0,   6, 256),   # L6 leaf-only half
    (384,  256, 96,  7,    0,   6, 0),     # L6 internal half
    (896,  128, 128, 6,    0,   5, 0),     # L5
    (1024, 32,  32,  5,    0,   4, 0),     # L4
]
_STATE_COLS = {7: L7P, 6: 512, 5: 128, 4: 32}


def _static_tree():
    idx = np.arange(N)[:, None] * K + 1 + np.arange(K)[None, :]
    mask = (idx < N).astype(F32)
    idx = np.where(idx < N, idx, 0).astype(np.int32)
    return idx, mask


_STATIC_IDX, _STATIC_MASK = _static_tree()


def _pack_weights(Wx, bx, Wh, bh):
    wx = np.zeros((KDIM, 4 * H), dtype=F32)  # cast to bf16 at return
    for g, rg in enumerate(GATE_MAP):
        wx[:D, H * g:H * (g + 1)] = np.asarray(Wx[rg], dtype=F32).T
        wx[D, H * g:H * (g + 1)] = np.asarray(bx[rg], dtype=F32) + np.asarray(bh[rg], dtype=F32)
    wh = np.zeros((H, 3 * H), dtype=F32)
    for g, rg in enumerate([0, 2, 3]):  # i, o, u
        wh[:, H * g:H * (g + 1)] = np.asarray(Wh[rg], dtype=F32).T
    whf = np.ascontiguousarray(np.asarray(Wh[1], dtype=F32).T)
    return wx.astype(BF16), wh, whf


def _pack_xt(xs, emb_table):
    X = np.asarray(emb_table, dtype=F32)[np.asarray(xs)]
    xts = []
    for cols, _, _ in _PLAN:
        xt = np.zeros((KDIM, XCOLS), dtype=F32)
        real = cols >= 0
        xt[:D, real] = X[cols[real]].T
        xt[D, real] = 1.0
        xts.append(xt.astype(BF16))
    return xts


def _sigmoid(x):
    return (1.0 / (1.0 + np.exp(-x))).astype(F32)


def _host_top(Hbuf, Cbuf, xs, emb_table, Wx, bx, Wh, bh):
    """Compute tree levels 3..0 (nodes 0..84) on the host, numpy fp32."""
    Wx = np.asarray(Wx, dtype=F32)
    bx = np.asarray(bx, dtype=F32)
    Wh = np.asarray(Wh, dtype=F32)
    bh = np.asarray(bh, dtype=F32)
    emb = np.asarray(emb_table, dtype=F32)
    xs = np.asarray(xs)
    for lo, hi in [(21, 85), (5, 21), (1, 5), (0, 1)]:
        ids = np.arange(lo, hi)
        Xl = emb[xs[ids]]                                   # [n, D]
        gx = np.einsum('ghd,nd->ngh', Wx, Xl).astype(F32) + bx
        cidx = ids[:, None] * K + 1 + np.arange(K)[None, :]  # all valid (< 341)
        Hc = Hbuf[cidx]
        Cc = Cbuf[cidx]
        hs = Hc.sum(1)
        ig = _sigmoid(gx[:, 0] + hs @ Wh[0].T + bh[0])
        og = _sigmoid(gx[:, 2] + hs @ Wh[2].T + bh[2])
        ug = np.tanh(gx[:, 3] + hs @ Wh[3].T + bh[3]).astype(F32)
        f = _sigmoid(gx[:, 1][:, None, :] + Hc @ Wh[1].T + bh[1])
        cc = ig * ug + (f * Cc).sum(1)
        hh = og * np.tanh(cc).astype(F32)
        Hbuf[ids] = hh
        Cbuf[ids] = cc
    return Hbuf[0]


def _log_softmax(x):
    m = np.max(x)
    e = np.exp(x - m)
    return (x - m - np.log(e.sum())).astype(F32)


def simulate_cores_numpy(inputs):
    """Numpy emulation of the exact device data layout & chunk schedule.

    Returns (Hbuf, Cbuf) filled for nodes [85, 341) — for validating the plan
    against the reference without hardware.
    """
    xs = np.asarray(inputs["xs"])
    wx, wh, whf = _pack_weights(inputs["Wx"], inputs["bx"], inputs["Wh"], inputs["bh"])
    xts = _pack_xt(xs, inputs["emb_table"])
    Hbuf = np.zeros((341, H), dtype=F32)
    Cbuf = np.zeros((341, H), dtype=F32)
    for c in range(NCORES):
        cols, wc, l4 = _PLAN[c]
        xt = xts[c]
        state_h = {lv: np.zeros((H, n), dtype=F32) for lv, n in _STATE_COLS.items()}
        state_c = {lv: np.zeros((H, n), dtype=F32) for lv, n in _STATE_COLS.items()}
        for (xoff, nc_, ip, child, coff, outlv, ooff) in _CHUNKS:
            xk = xt[:KUSE, xoff:xoff + nc_].astype(F32)         # [301, nc]
            G = wx[:KUSE].astype(F32).T @ xk                    # [1024, nc]
            gi = G[0:H]
            go = G[H:2 * H]
            gu = G[2 * H:3 * H]
            gf = G[3 * H:4 * H]
            if ip > 0:
                ch_h = state_h[child][:, coff:coff + 4 * ip]    # [H, 4ip]
                ch_c = state_c[child][:, coff:coff + 4 * ip]
                hs = ch_h.reshape(H, ip, K).sum(axis=2)         # [H, ip]
                A = wh.T @ hs                                   # [768, ip]
                gi[:, :ip] += A[0:H]
                go[:, :ip] += A[H:2 * H]
                gu[:, :ip] += A[2 * H:3 * H]
                Fp = whf.T @ ch_h                               # [H, 4ip]
                FA = Fp + np.repeat(gf[:, :ip], K, axis=1)
                FS = _sigmoid(FA) * ch_c
                csum = FS.reshape(H, ip, K).sum(axis=2)
            ig = _sigmoid(gi)
            og = _sigmoid(go)
            ug = np.tanh(gu).astype(F32)
            cc = ig * ug
            if ip > 0:
                cc[:, :ip] += csum
            hh = og * np.tanh(cc).astype(F32)
            state_h[outlv][:, ooff:ooff + nc_] = hh
            state_c[outlv][:, ooff:ooff + nc_] = cc
        Hbuf[l4] = state_h[4].T
        Cbuf[l4] = state_c[4].T
    return Hbuf, Cbuf


# ----------------------------------------------------------------------------
# Bass device program
# ----------------------------------------------------------------------------

_COMPILED = None


def _build_device_program():
    import contextlib

    import concourse.bacc as bacc
    import concourse.tile as tile
    import concourse.mybir as mybir

    f32 = mybir.dt.float32
    f32r = mybir.dt.float32r
    bf16 = mybir.dt.bfloat16
    Sig = mybir.ActivationFunctionType.Sigmoid
    Tanh = mybir.ActivationFunctionType.Tanh

    nc = bacc.Bacc("TRN2", target_bir_lowering=False, debug=False,
                   num_devices=NCORES)

    def mm(out, lhsT, rhs, **kw):
        # float32r operands: same fp32 bytes, single-pass reduced-precision
        # multiply (vs fp32's two half-speed passes + double weight load).
        nc.tensor.matmul(out, lhsT, rhs, **kw)

    xt_d = nc.dram_tensor("xt", [KDIM, XCOLS], bf16, kind="ExternalInput")
    wx_d = nc.dram_tensor("wx", [KDIM, 4 * H], bf16, kind="ExternalInput")
    wh_d = nc.dram_tensor("wh", [H, 3 * H], f32r, kind="ExternalInput")
    whf_d = nc.dram_tensor("whf", [H, H], f32r, kind="ExternalInput")
    out_h_d = nc.dram_tensor("out_h", [128, 2, 32], f32r, kind="ExternalOutput")
    out_c_d = nc.dram_tensor("out_c", [128, 2, 32], f32, kind="ExternalOutput")

    krows = [(0, 128), (128, 256), (256, KUSE)]
    RANGES = [(0, 512), (512, 1024), (1024, XCOLS)]

    with tile.TileContext(nc) as tc:
        with contextlib.ExitStack() as ctx:
            inp = ctx.enter_context(tc.tile_pool(name="inp", bufs=1))
            st = ctx.enter_context(tc.tile_pool(name="state", bufs=1))
            wk = ctx.enter_context(tc.tile_pool(name="work", bufs=2))
            fwk = ctx.enter_context(tc.tile_pool(name="fwork", bufs=3))
            ps = ctx.enter_context(
                tc.tile_pool(name="psum", bufs=2, space="PSUM"))

            # --- inputs to SBUF, spread across otherwise-idle engines; xt is
            # split into chunk-aligned column ranges so the first level can
            # start as soon as its columns land.
            xt_s = []
            wx_s = []
            for k, (r0, r1) in enumerate(krows[:2] + [(256, 304)]):
                t = inp.tile([r1 - r0, 4 * H], bf16, tag=f"wx{k}", name=f"wx{k}")
                wx_s.append(t)
            # wx on gpsimd in column quarters, k-interleaved, so the first
            # gate's weights land quickly
            for q in range(4):
                for k, (r0, r1) in enumerate(krows[:2] + [(256, 304)]):
                    nc.gpsimd.dma_start(
                        out=wx_s[k][:, 256 * q:256 * (q + 1)],
                        in_=wx_d[r0:r1, 256 * q:256 * (q + 1)])
            wh_s = []
            whf_s = []
            for k, (r0, r1) in enumerate([(0, 128), (128, 256)]):
                t = inp.tile([128, 3 * H], f32r, tag=f"wh{k}", name=f"wh{k}")
                nc.scalar.dma_start(out=t[:], in_=wh_d[r0:r1, :])
                wh_s.append(t)
                t = inp.tile([128, H], f32r, tag=f"whf{k}", name=f"whf{k}")
                nc.scalar.dma_start(out=t[:], in_=whf_d[r0:r1, :])
                whf_s.append(t)
            for k, (r0, r1) in enumerate(krows[:2] + [(256, 304)]):
                t = inp.tile([r1 - r0, XCOLS], bf16, tag=f"xt{k}", name=f"xt{k}")
                xt_s.append(t)
            # xt on sync, range-major so the first level's columns land first
            for (a, b) in RANGES:
                for k, (r0, r1) in enumerate(krows[:2] + [(256, 304)]):
                    nc.sync.dma_start(out=xt_s[k][:, a:b], in_=xt_d[r0:r1, a:b])

            # --- persistent state + gx tiles
            SH = {lv: st.tile([128, 2, n], f32r, tag=f"h{lv}", name=f"sh{lv}")
                  for lv, n in _STATE_COLS.items()}
            SC = {lv: st.tile([128, 2, n], f32, tag=f"c{lv}", name=f"sc{lv}")
                  for lv, n in _STATE_COLS.items()}
            GX = [st.tile([128, 2, XCOLS], f32, tag=f"gx{g}", name=f"gx{g}")
                  for g in range(4)]

            # --- phase 1: all x-side gate projections in one weight-stationary
            # sweep (minimal LDWEIGHTS: 24 distinct weight tiles, loaded once),
            # drained PSUM -> SBUF by DMA.
            for m in range(8):
                g, phi = divmod(m, 2)
                col = H * g + 128 * phi
                ptiles = [
                    ps.tile([128, b - a], f32, tag="gx", name=f"pgx{m}_{i}",
                            bufs=4)
                    for i, (a, b) in enumerate(RANGES)
                ]
                for k in range(3):
                    r0, r1 = krows[k]
                    for i, (a, b) in enumerate(RANGES):
                        mm(ptiles[i][:],
                           wx_s[k][0:r1 - r0, col:col + 128],
                           xt_s[k][0:r1 - r0, a:b],
                           start=(k == 0), stop=(k == 2))
                for i, (a, b) in enumerate(RANGES):
                    if (m + i) % 2 == 0:
                        nc.scalar.copy(GX[g][:, phi, a:b], ptiles[i][:])
                    else:
                        nc.vector.tensor_copy(GX[g][:, phi, a:b], ptiles[i][:])

            # --- phase 2: levels bottom-up in chunks
            for (xoff, cn, ip, child, coff, outlv, ooff) in _CHUNKS:
                hs = None
                if ip > 0:
                    # hs = sum of the 4 child h columns per node (on the
                    # otherwise-idle gpsimd engine)
                    hs = wk.tile([128, 2, ip], f32r, tag="hs", name="hs")
                    for phi in range(2):
                        cv = SH[child][:, phi, coff:coff + 4 * ip].rearrange(
                            "p (n k) -> p n k", k=K)
                        dst = hs[:, phi, :]
                        nc.gpsimd.tensor_add(dst, cv[:, :, 0], cv[:, :, 1])
                        nc.gpsimd.tensor_add(dst, dst, cv[:, :, 2])
                        nc.gpsimd.tensor_add(dst, dst, cv[:, :, 3])

                gates = []
                for gi_, func in ((0, Sig), (1, Sig), (2, Tanh)):
                    G = wk.tile([128, 2, cn], f32, tag=f"g{gi_}", name=f"g{gi_}")
                    if ip > 0:
                        P = ps.tile([128, 2, ip], f32, tag="pa", name=f"pa{gi_}",
                                    bufs=2)
                        for phi in range(2):
                            for k in range(2):
                                mm(P[:, phi, :],
                                   wh_s[k][:, H * gi_ + 128 * phi:
                                           H * gi_ + 128 * phi + 128],
                                   hs[:, k, 0:ip],
                                   start=(k == 0), stop=(k == 1))
                        nc.vector.tensor_add(
                            P[:], P[:], GX[gi_][:, :, xoff:xoff + ip])
                        nc.scalar.activation(G[:, :, 0:ip], P[:], func)
                        if cn > ip:
                            nc.scalar.activation(
                                G[:, :, ip:cn],
                                GX[gi_][:, :, xoff + ip:xoff + cn], func)
                    else:
                        nc.scalar.activation(
                            G[:], GX[gi_][:, :, xoff:xoff + cn], func)
                    gates.append(G)
                IG, OG, UG = gates

                csum = None
                if ip > 0:
                    # f = sigmoid(gf + Whf @ h_child), per child; then
                    # csum = sum_k f_k * c_child_k
                    csum = wk.tile([128, 2, ip], f32, tag="csum", name="csum")
                    for phi in range(2):
                        Pfc = ps.tile([128, 4 * ip], f32, tag="pf",
                                      name=f"pfc{phi}")
                        for k in range(2):
                            mm(Pfc[:],
                               whf_s[k][:, 128 * phi:128 * phi + 128],
                               SH[child][:, k, coff:coff + 4 * ip],
                               start=(k == 0), stop=(k == 1))
                        pv = Pfc.rearrange("p (n k) -> p n k", k=K)
                        gfb = GX[3][:, phi, xoff:xoff + ip][:, :, None]
                        nc.vector.tensor_add(
                            pv, pv, gfb.broadcast_to([128, ip, K]))
                        FS = fwk.tile([128, 4 * ip], f32, tag="fs", name="fs")
                        nc.scalar.activation(FS[:], Pfc[:], Sig)
                        nc.vector.tensor_mul(
                            FS[:], FS[:], SC[child][:, phi, coff:coff + 4 * ip])
                        sv = FS.rearrange("p (n k) -> p n k", k=K)
                        dst = csum[:, phi, :]
                        nc.gpsimd.tensor_add(dst, sv[:, :, 0], sv[:, :, 1])
                        nc.gpsimd.tensor_add(dst, dst, sv[:, :, 2])
                        nc.gpsimd.tensor_add(dst, dst, sv[:, :, 3])

                # c = ig*ug (+ csum on internal cols); h = og*tanh(c)
                Cdst = SC[outlv][:, :, ooff:ooff + cn]
                nc.vector.tensor_mul(Cdst, IG[:], UG[:])
                if ip > 0:
                    nc.vector.tensor_add(
                        SC[outlv][:, :, ooff:ooff + ip],
                        SC[outlv][:, :, ooff:ooff + ip],
                        csum[:],
                    )
                TC = wk.tile([128, 2, cn], f32, tag="tc", name="tc")
                nc.scalar.activation(TC[:], Cdst, Tanh)
                nc.vector.tensor_mul(
                    SH[outlv][:, :, ooff:ooff + cn], OG[:], TC[:])

            nc.sync.dma_start(out=out_h_d[:], in_=SH[4][:])
            nc.sync.dma_start(out=out_c_d[:], in_=SC[4][:])

    nc.compile()
    return nc


def _get_compiled():
    global _COMPILED
    if _COMPILED is None:
        _COMPILED = _build_device_program()
    return _COMPILED


def _numpy_fallback(xs, child_idx, child_mask, emb_table, Wx, bx, Wh, bh,
                    Wout, bout):
    """Exact sequential scan in numpy; only used if the tree is not the
    expected static 4-ary heap."""
    X = np.asarray(emb_table, dtype=F32)[np.asarray(xs)]
    Wx = np.asarray(Wx, dtype=F32)
    Wh = np.asarray(Wh, dtype=F32)
    bx = np.asarray(bx, dtype=F32)
    bh = np.asarray(bh, dtype=F32)
    gx = np.einsum('ghd,nd->ngh', Wx, X).astype(F32) + bx
    Hb = np.zeros((N, H), dtype=F32)
    Cb = np.zeros((N, H), dtype=F32)
    ci = np.asarray(child_idx)
    cm = np.asarray(child_mask, dtype=F32)
    for i in range(N - 1, -1, -1):
        idx = ci[i]
        m = cm[i][:, None]
        Hc = Hb[idx] * m
        Cc = Cb[idx] * m
        hs = Hc.sum(0)
        g = gx[i]
        ig = _sigmoid(g[0] + Wh[0] @ hs + bh[0])
        og = _sigmoid(g[2] + Wh[2] @ hs + bh[2])
        ug = np.tanh(g[3] + Wh[3] @ hs + bh[3]).astype(F32)
        f = _sigmoid(g[1] + Hc @ Wh[1].T + bh[1])
        c = ig * ug + (f * Cc).sum(0)
        Hb[i] = og * np.tanh(c).astype(F32)
        Cb[i] = c
    logits = np.asarray(Wout, dtype=F32) @ Hb[0] + np.asarray(bout, dtype=F32)
    return _log_softmax(logits)


def kernel(xs, child_idx, child_mask, emb_table, Wx, bx, Wh, bh, Wout, bout):
    xs = np.asarray(xs)
    if not (np.array_equal(np.asarray(child_idx), _STATIC_IDX)
            and np.array_equal(np.asarray(child_mask, dtype=F32), _STATIC_MASK)):
        return _numpy_fallback(xs, child_idx, child_mask, emb_table, Wx, bx,
                               Wh, bh, Wout, bout)

    from concourse.bass_utils import run_bass_kernel_spmd

    wx, wh, whf = _pack_weights(Wx, bx, Wh, bh)
    xts = _pack_xt(xs, emb_table)
    in_maps = [
        {"xt": xts[c], "wx": wx, "wh": wh, "whf": whf} for c in range(NCORES)
    ]
    nc = _get_compiled()
    res = run_bass_kernel_spmd(nc, in_maps, core_ids=list(range(NCORES)))

    Hbuf = np.zeros((341, H), dtype=F32)
    Cbuf = np.zeros((341, H), dtype=F32)
    for c in range(NCORES):
        _, _, l4 = _PLAN[c]
        oh = res.results[c]["out_h"]   # [128, 2, 32]
        oc = res.results[c]["out_c"]
        Hbuf[l4] = np.concatenate([oh[:, 0, :], oh[:, 1, :]], axis=0).T
        Cbuf[l4] = np.concatenate([oc[:, 0, :], oc[:, 1, :]], axis=0).T

    h0 = _host_top(Hbuf, Cbuf, xs, emb_table, Wx, bx, Wh, bh)
    logits = np.asarray(Wout, dtype=F32) @ h0 + np.asarray(bout, dtype=F32)
    return _log_softmax(logits)



# revision 6
# speedup vs baseline: 1.5817x; 1.5817x over previous
"""ChildSum TreeLSTM (N=8192, 4-ary heap tree, H=256, D=300) on 8 trn2 cores.

Design (v2 — see kernel_baseline.py.bak for the prior version)
--------------------------------------------------------------
The static tree is processed level-by-level bottom-up. Each core owns 32
level-4 subtrees; the device computes levels 7, 6, 5 and ships the 1024
level-5 (h, c) states back; the host finishes levels 4..0 (341 nodes).

Per-core column layout (XCOLS = 1024):
  [ L7 child-major 0:384 | L6-leaf 384:800 | L6-int 800:896 | L5 896:1024 ]
The last 224 columns are the "internal" nodes (the only ones with children).

Key device tricks:
- x-side gate projections (phase 1) go straight to PSUM; leaf activations
  read PSUM directly (no drain copies). For the 224 internal columns the
  i/o/u projections stay RESIDENT in PSUM and the phase-2 Wh@hs matmuls
  accumulate onto them in place (start=False), so the gate pre-activations
  never touch SBUF.
- the per-child forget-gate bias gf is added with an identity matmul that
  broadcasts gf into the f PSUM tile (PE work instead of 1x-mode DVE adds).
- L7 states are stored child-major so the 4-child h/c reductions are
  unit-stride bf16 2x-mode adds.
- everything on device is bf16 (FWL weight loads, 2x DVE) with fp32 PSUM.
- activation tables (sigmoid/tanh) are preloaded at t=0 with dummy ACTs.
"""

import numpy as np
import ml_dtypes

BF16 = ml_dtypes.bfloat16
F32 = np.float32

N = 8192
H = 256
D = 300
K = 4
OUT = 4
NCORES = 8

L7P = 384            # L7 columns (child-major: plane c holds child c of l6[j])
NL6 = 512            # L6 columns per core
NL5 = 128            # L5 columns per core
IPMAX = 96           # internal L6 columns (l6[:96]; pads self-compute as leaves)
NINT = IPMAX + NL5   # 224 internal columns [L6i | L5]
XCOLS = L7P + NL6 + NL5   # 1024
KDIM = 304           # xt rows: 300 emb + 1 ones + 3 pad
KROWS = [(0, 128), (128, 256), (256, KDIM)]

GATE_MAP = [0, 2, 3, 1]  # our gate order [i, o, u, f] -> reference indices

NHOST = 341          # host computes nodes [0, 341); device supplies [341, 1365)


def _build_plan():
    """Assign the 256 level-4 subtrees to 8 cores; build per-core column maps."""
    full = list(range(85, 127))                               # w(u)=16 each
    lights = list(range(128, 341))                            # w(u)=0
    heavy_counts = [6, 6, 5, 5, 5, 5, 5, 5]                   # sums to 42
    light_counts = [26, 26, 26, 27, 27, 27, 27, 27]           # sums to 213
    cores = []
    hpos = lpos = 0
    for c in range(NCORES):
        hs = full[hpos:hpos + heavy_counts[c]]
        hpos += heavy_counts[c]
        if c == 2:
            hs = hs + [127]                                   # w(127)=11
        ls = lights[lpos:lpos + light_counts[c]]
        lpos += light_counts[c]
        cores.append(sorted(hs + ls))
    assert sorted(u for cs in cores for u in cs) == list(range(85, 341))

    plan = []
    for c in range(NCORES):
        l4 = cores[c]
        assert len(l4) == 32
        l5 = [4 * u + 1 + k for u in l4 for k in range(K)]
        l6 = [4 * v + 1 + k for v in l5 for k in range(K)]
        wc = sum(1 for x in l6 if x < 2048)
        assert wc <= IPMAX
        # internal l6 nodes must be a prefix of l6
        assert all(x < 2048 for x in l6[:wc])
        assert all(x >= 2048 for x in l6[wc:])
        # L7 child-major: col (ch*96 + j) = child ch of l6[j] (j < 96)
        l7 = np.full((K, IPMAX), -1, dtype=np.int64)
        for j in range(min(wc, IPMAX)):
            x = l6[j]
            for ch in range(K):
                cc = 4 * x + 1 + ch
                l7[ch, j] = cc if cc < N else -1
        # xt column order: [L7 | L6-leaf | L6-int | L5] — internal block last
        cols = np.concatenate([l7.reshape(-1),
                               np.array(l6[IPMAX:] + l6[:IPMAX] + l5,
                                        dtype=np.int64)])
        assert cols.shape == (XCOLS,)
        plan.append((cols, wc, np.array(l5, dtype=np.int64)))
    return plan


_PLAN = _build_plan()


def _static_tree():
    idx = np.arange(N)[:, None] * K + 1 + np.arange(K)[None, :]
    mask = (idx < N).astype(F32)
    idx = np.where(idx < N, idx, 0).astype(np.int32)
    return idx, mask


_STATIC_IDX, _STATIC_MASK = _static_tree()


def _pack_weights(Wx, bx, Wh, bh):
    wx = np.zeros((KDIM, 4 * H), dtype=F32)
    for g, rg in enumerate(GATE_MAP):
        wx[:D, H * g:H * (g + 1)] = np.asarray(Wx[rg], dtype=F32).T
        wx[D, H * g:H * (g + 1)] = (np.asarray(bx[rg], dtype=F32)
                                    + np.asarray(bh[rg], dtype=F32))
    wh = np.zeros((H, 3 * H), dtype=F32)
    for g, rg in enumerate([0, 2, 3]):  # i, o, u
        wh[:, H * g:H * (g + 1)] = np.asarray(Wh[rg], dtype=F32).T
    whf = np.ascontiguousarray(np.asarray(Wh[1], dtype=F32).T)
    ident = np.eye(128, dtype=F32)
    return (wx.astype(BF16), wh.astype(BF16), whf.astype(BF16),
            ident.astype(BF16))


def _pack_xt(xs, emb_table):
    X = np.asarray(emb_table, dtype=F32)[np.asarray(xs)]
    xts = []
    for cols, _, _ in _PLAN:
        xt = np.zeros((KDIM, XCOLS), dtype=F32)
        real = cols >= 0
        xt[:D, real] = X[cols[real]].T
        xt[D, real] = 1.0
        xts.append(xt.astype(BF16))
    return xts


def _sigmoid(x):
    return (1.0 / (1.0 + np.exp(-x))).astype(F32)


def _log_softmax(x):
    m = np.max(x)
    e = np.exp(x - m)
    return (x - m - np.log(e.sum())).astype(F32)


def _host_top(Hbuf, Cbuf, xs, emb_table, Wx, bx, Wh, bh):
    """Compute tree levels 4..0 (nodes 0..340) on the host in fp32 numpy."""
    Wx = np.asarray(Wx, dtype=F32)
    bx = np.asarray(bx, dtype=F32)
    Wh = np.asarray(Wh, dtype=F32)
    bh = np.asarray(bh, dtype=F32)
    emb = np.asarray(emb_table, dtype=F32)
    xs = np.asarray(xs)
    for lo, hi in [(85, 341), (21, 85), (5, 21), (1, 5), (0, 1)]:
        ids = np.arange(lo, hi)
        Xl = emb[xs[ids]]
        gx = np.einsum('ghd,nd->ngh', Wx, Xl).astype(F32) + bx
        cidx = ids[:, None] * K + 1 + np.arange(K)[None, :]
        Hc = Hbuf[cidx]
        Cc = Cbuf[cidx]
        hsum = Hc.sum(1)
        ig = _sigmoid(gx[:, 0] + hsum @ Wh[0].T + bh[0])
        og = _sigmoid(gx[:, 2] + hsum @ Wh[2].T + bh[2])
        ug = np.tanh(gx[:, 3] + hsum @ Wh[3].T + bh[3]).astype(F32)
        f = _sigmoid(gx[:, 1][:, None, :] + Hc @ Wh[1].T + bh[1])
        cc = ig * ug + (f * Cc).sum(1)
        hh = og * np.tanh(cc).astype(F32)
        Hbuf[ids] = hh
        Cbuf[ids] = cc
    return Hbuf[0]


def simulate_cores_numpy(inputs):
    """Numpy emulation of the device data layout & schedule (fp32 math).

    Returns (Hbuf, Cbuf) [1365, H] filled for nodes [341, 1365) — validates
    the plan/layout without hardware.
    """
    xs = np.asarray(inputs["xs"])
    wx, wh, whf, _ = _pack_weights(inputs["Wx"], inputs["bx"],
                                   inputs["Wh"], inputs["bh"])
    wx = wx.astype(F32)
    wh = wh.astype(F32)
    whf = whf.astype(F32)
    xts = _pack_xt(xs, inputs["emb_table"])
    Hbuf = np.zeros((1365, H), dtype=F32)
    Cbuf = np.zeros((1365, H), dtype=F32)
    for c in range(NCORES):
        cols, wc, l5 = _PLAN[c]
        xt = xts[c].astype(F32)
        G = wx[:301].T @ xt[:301]                    # [1024, XCOLS]
        gi, go, gu, gf = (G[0:H], G[H:2*H], G[2*H:3*H], G[3*H:4*H])

        def leaf(sl):
            cc = _sigmoid(gi[:, sl]) * np.tanh(gu[:, sl]).astype(F32)
            hh = _sigmoid(go[:, sl]) * np.tanh(cc).astype(F32)
            return hh, cc

        H7, C7 = leaf(slice(0, L7P))                 # [H, 384] child-major
        H6 = np.zeros((H, NL6), dtype=F32)
        C6 = np.zeros((H, NL6), dtype=F32)
        H6[:, 96:], C6[:, 96:] = leaf(slice(L7P, L7P + 416))

        def internal(ch_h, ch_c, q0, ip, child_major):
            sl = slice(800 + q0, 800 + q0 + ip)   # internal block gx cols
            if child_major:
                hs = ch_h.reshape(H, K, ip).sum(1)
                gfr = np.tile(gf[:, sl], (1, K))                 # [H, K*ip]
            else:
                hs = ch_h.reshape(H, ip, K).sum(2)
                gfr = np.repeat(gf[:, sl], K, axis=1)
            A = wh.T @ hs                                        # [768, ip]
            ig = _sigmoid(gi[:, sl] + A[0:H])
            og = _sigmoid(go[:, sl] + A[H:2*H])
            ug = np.tanh(gu[:, sl] + A[2*H:3*H]).astype(F32)
            FA = whf.T @ ch_h + gfr
            FS = _sigmoid(FA) * ch_c
            if child_major:
                csum = FS.reshape(H, K, ip).sum(1)
            else:
                csum = FS.reshape(H, ip, K).sum(2)
            cc = ig * ug + csum
            hh = og * np.tanh(cc).astype(F32)
            return hh, cc

        H6[:, :96], C6[:, :96] = internal(H7, C7, 0, 96, True)
        H5, C5 = internal(H6, C6, 96, NL5, False)
        Hbuf[l5] = H5.T
        Cbuf[l5] = C5.T
    return Hbuf, Cbuf


# ----------------------------------------------------------------------------
# Bass device program
# ----------------------------------------------------------------------------

_COMPILED = None


def _build_device_program():
    import contextlib

    import concourse.bacc as bacc
    import concourse.tile as tile
    import concourse.mybir as mybir

    f32 = mybir.dt.float32
    bf16 = mybir.dt.bfloat16
    Sig = mybir.ActivationFunctionType.Sigmoid
    Tanh = mybir.ActivationFunctionType.Tanh

    nc = bacc.Bacc("TRN2", target_bir_lowering=False, debug=False,
                   num_devices=NCORES)

    xt_d = nc.dram_tensor("xt", [KDIM, XCOLS], bf16, kind="ExternalInput")
    wx_d = nc.dram_tensor("wx", [KDIM, 4 * H], bf16, kind="ExternalInput")
    wh_d = nc.dram_tensor("wh", [H, 3 * H], bf16, kind="ExternalInput")
    whf_d = nc.dram_tensor("whf", [H, H], bf16, kind="ExternalInput")
    id_d = nc.dram_tensor("ident", [128, 128], bf16, kind="ExternalInput")
    out_h_d = nc.dram_tensor("out_h", [128, 2, NL5], bf16,
                             kind="ExternalOutput")
    out_c_d = nc.dram_tensor("out_c", [128, 2, NL5], bf16,
                             kind="ExternalOutput")

    # column ranges of xt: R0 = L7 leaves, R1 = L6 leaves, RI = internal
    R0 = (0, L7P)
    R1 = (L7P, L7P + 416)
    RI = (800, 1024)

    with tile.TileContext(nc) as tc:
        with contextlib.ExitStack() as ctx:
            inp = ctx.enter_context(tc.tile_pool(name="inp", bufs=1))
            st = ctx.enter_context(tc.tile_pool(name="state", bufs=1))
            wk = ctx.enter_context(tc.tile_pool(name="work", bufs=2))
            pres = ctx.enter_context(
                tc.tile_pool(name="pres", bufs=1, space="PSUM"))
            pstr = ctx.enter_context(
                tc.tile_pool(name="pstr", bufs=2, space="PSUM"))
            pfres = ctx.enter_context(
                tc.tile_pool(name="pfres", bufs=1, space="PSUM"))

            # ---- activation-table preload (sigmoid set incl. tanh) at t=0
            scr = wk.tile([128, 8], f32, tag="scr", name="scr")
            nc.vector.memset(scr[:], 0.0)
            nc.scalar.activation(scr[:], scr[:], Sig)
            nc.scalar.activation(scr[:], scr[:], Tanh)

            # ---- input SBUF tiles
            xt_s = []
            wx_s = []
            for k, (r0, r1) in enumerate(KROWS):
                xt_s.append(inp.tile([r1 - r0, XCOLS], bf16, tag=f"xt{k}",
                                     name=f"xt{k}"))
                wx_s.append(inp.tile([r1 - r0, 4 * H], bf16, tag=f"wx{k}",
                                     name=f"wx{k}"))
            wh_s = []
            whf_s = []
            for k in range(2):
                wh_s.append(inp.tile([128, 3 * H], bf16, tag=f"wh{k}",
                                     name=f"wh{k}"))
                whf_s.append(inp.tile([128, H], bf16, tag=f"whf{k}",
                                      name=f"whf{k}"))
            id_s = inp.tile([128, 128], bf16, tag="ident", name="ident")

            # ---- DMA in, priority order
            # queue A (sync): xt by (range, k) — R0 first, then RI, then R1
            for (a, b) in (R0, RI, R1):
                for k, (r0, r1) in enumerate(KROWS):
                    nc.sync.dma_start(out=xt_s[k][:, a:b],
                                      in_=xt_d[r0:r1, a:b])
            # queue B (scalar): wx by (gate, k) — i, o, u, f
            for g in range(4):
                for k, (r0, r1) in enumerate(KROWS):
                    nc.scalar.dma_start(
                        out=wx_s[k][:, 256 * g:256 * (g + 1)],
                        in_=wx_d[r0:r1, 256 * g:256 * (g + 1)])
            # queue C (gpsimd): h-side weights + identity (needed ~6us in)
            for k in range(2):
                nc.gpsimd.dma_start(out=wh_s[k][:], in_=wh_d[128*k:128*(k+1), :])
                nc.gpsimd.dma_start(out=whf_s[k][:],
                                    in_=whf_d[128*k:128*(k+1), :])
            nc.gpsimd.dma_start(out=id_s[:], in_=id_d[:, :])

            # ---- persistent state tiles (bf16)
            SH7 = st.tile([128, 2, L7P], bf16, tag="sh7", name="sh7")
            SC7 = st.tile([128, 2, L7P], bf16, tag="sc7", name="sc7")
            SH6 = st.tile([128, 2, NL6], bf16, tag="sh6", name="sh6")
            SC6 = st.tile([128, 2, NL6], bf16, tag="sc6", name="sc6")
            SH5 = st.tile([128, 2, NL5], bf16, tag="sh5", name="sh5")
            SC5 = st.tile([128, 2, NL5], bf16, tag="sc5", name="sc5")

            # ---- resident PSUM for internal-column gate pre-activations
            # res_io: [g(i=0,o=1), phi, 256] — 2 banks; res_u: 1 bank
            res_io = pres.tile([128, 2, 2, 256], f32, tag="rio", name="rio")
            res_u = pres.tile([128, 2, 256], f32, tag="ru", name="ru")
            gf_ps = pfres.tile([128, 2, 256], f32, tag="rf", name="rf")
            gf_s = st.tile([128, 2, NINT], bf16, tag="gfs", name="gfs")

            def mm(out, lhsT, rhs, start, stop):
                nc.tensor.matmul(out, lhsT, rhs, start=start, stop=stop,
                                 skip_group_check=True)

            # --- phase-1 helper: one gate over cols [a,b) into tile P
            def p1_gate(P, g, a, b, dst_off=0):
                n = b - a
                for phi in range(2):
                    for k, (r0, r1) in enumerate(KROWS):
                        mm(P[:, phi, dst_off:dst_off + n],
                           wx_s[k][:, 256 * g + 128 * phi:
                                   256 * g + 128 * phi + 128],
                           xt_s[k][:, a:b],
                           start=(k == 0), stop=(k == 2))

            # --- phase-1 into resident banks (internal cols, all 4 gates)
            def p1_resident():
                a, b = RI
                n = b - a  # 224
                for gi_, g in ((0, 0), (1, 1)):  # i -> res_io[:,0], o -> [:,1]
                    for phi in range(2):
                        for k in range(3):
                            mm(res_io[:, gi_, phi, 0:n],
                               wx_s[k][:, 256 * g + 128 * phi:
                                       256 * g + 128 * phi + 128],
                               xt_s[k][:, a:b],
                               start=(gi_ == 0 and phi == 0 and k == 0) or
                                     (gi_ == 1 and phi == 0 and k == 0),
                               stop=False)
                for phi in range(2):
                    for k in range(3):
                        mm(res_u[:, phi, 0:n],
                           wx_s[k][:, 512 + 128 * phi:512 + 128 * phi + 128],
                           xt_s[k][:, a:b],
                           start=(phi == 0 and k == 0), stop=False)
                for phi in range(2):
                    for k in range(3):
                        mm(gf_ps[:, phi, 0:n],
                           wx_s[k][:, 768 + 128 * phi:768 + 128 * phi + 128],
                           xt_s[k][:, a:b],
                           start=(phi == 0 and k == 0),
                           stop=(phi == 1 and k == 2))
                # gf to SBUF bf16; the gf_ps bank is then free
                nc.vector.tensor_copy(gf_s[:], gf_ps[:, :, 0:NINT])

            # --- leaf elementwise: PSUM gate tiles -> states
            def leaf_states(Pi, Po, Pu, n, SH, SC, off):
                GI = wk.tile([128, 2, n], bf16, tag="gi", name=f"gi{off}")
                GO = wk.tile([128, 2, n], bf16, tag="go", name=f"go{off}")
                GU = wk.tile([128, 2, n], bf16, tag="gu", name=f"gu{off}")
                nc.scalar.activation(GI[:], Pi[:, :, 0:n], Sig)
                nc.scalar.activation(GO[:], Po[:, :, 0:n], Sig)
                nc.scalar.activation(GU[:], Pu[:, :, 0:n], Tanh)
                Cd = SC[:, :, off:off + n]
                nc.vector.tensor_mul(Cd, GI[:], GU[:])
                TC = wk.tile([128, 2, n], bf16, tag="tc", name=f"tc{off}")
                nc.scalar.activation(TC[:], Cd, Tanh)
                nc.vector.tensor_mul(SH[:, :, off:off + n], GO[:], TC[:])

            def leaf_range(a, b, SH, SC, off):
                n = b - a
                Pi = pstr.tile([128, 2, 512], f32, tag="lps", name=f"pi{a}")
                p1_gate(Pi, 0, a, b)
                Po = pstr.tile([128, 2, 512], f32, tag="lps", name=f"po{a}")
                p1_gate(Po, 1, a, b)
                Pu = pstr.tile([128, 2, 512], f32, tag="lps", name=f"pu{a}")
                p1_gate(Pu, 2, a, b)
                leaf_states(Pi, Po, Pu, n, SH, SC, off)

            # --- internal chunk: children states -> states at level above
            def chunk(q0, ip, SHc, SCc, ccols, child_major, SHo, SCo, off,
                      last):
                # f path first: FA = Whf @ ch_h + gf (identity matmul)
                Pf = pstr.tile([128, 2, 512], f32, tag="lps", name=f"pf{q0}")
                nf = K * ip
                for phi in range(2):
                    for k in range(2):
                        mm(Pf[:, phi, 0:nf],
                           whf_s[k][:, 128 * phi:128 * phi + 128],
                           SHc[:, k, 0:ccols],
                           start=(k == 0), stop=False)
                    gslice = gf_s[:, phi, q0:q0 + ip]
                    if child_major:
                        gbr = gslice[:, None, :].broadcast_to([128, K, ip])
                    else:
                        gbr = gslice[:, :, None].broadcast_to([128, ip, K])
                    mm(Pf[:, phi, 0:nf], id_s[:, 0:128], gbr,
                       start=False, stop=True)
                # hs = sum of 4 children
                hs = wk.tile([128, 2, ip], bf16, tag="hs", name=f"hs{q0}")
                if child_major:
                    cv = SHc.rearrange("p t (c j) -> p t c j", c=K)
                    A = wk.tile([128, 2, ip], bf16, tag="ha", name=f"ha{q0}")
                    nc.gpsimd.tensor_add(A[:], cv[:, :, 0, :], cv[:, :, 1, :])
                    B = wk.tile([128, 2, ip], bf16, tag="hb", name=f"hb{q0}")
                    nc.gpsimd.tensor_add(B[:], cv[:, :, 2, :], cv[:, :, 3, :])
                    nc.vector.tensor_add(hs[:], A[:], B[:])
                else:
                    cv = SHc.rearrange("p t (j c) -> p t j c", c=2)
                    s = wk.tile([128, 2, 2 * ip], bf16, tag="ha",
                                name=f"ha{q0}")
                    nc.gpsimd.tensor_add(s[:], cv[:, :, :, 0], cv[:, :, :, 1])
                    sv = s.rearrange("p t (j c) -> p t j c", c=2)
                    nc.vector.tensor_add(hs[:], sv[:, :, :, 0], sv[:, :, :, 1])
                # i/o/u h-side accumulate onto resident banks
                for gi_ in range(2):  # i, o
                    for phi in range(2):
                        for k in range(2):
                            mm(res_io[:, gi_, phi, q0:q0 + ip],
                               wh_s[k][:, 256 * gi_ + 128 * phi:
                                       256 * gi_ + 128 * phi + 128],
                               hs[:, k, 0:ip],
                               start=False, stop=(last and k == 1))
                for phi in range(2):
                    for k in range(2):
                        mm(res_u[:, phi, q0:q0 + ip],
                           wh_s[k][:, 512 + 128 * phi:512 + 128 * phi + 128],
                           hs[:, k, 0:ip],
                           start=False, stop=(last and k == 1))
                # activations
                SF = wk.tile([128, 2, nf], bf16, tag="sf", name=f"sf{q0}")
                nc.scalar.activation(SF[:], Pf[:, :, 0:nf], Sig)
                Gio = wk.tile([128, 2, 2, ip], bf16, tag="gio",
                              name=f"gio{q0}")
                nc.scalar.activation(Gio[:], res_io[:, :, :, q0:q0 + ip], Sig)
                GU = wk.tile([128, 2, ip], bf16, tag="gu", name=f"cgu{q0}")
                nc.scalar.activation(GU[:], res_u[:, :, q0:q0 + ip], Tanh)
                # FS = sigmoid(FA) * c_child ; csum = sum over 4 children
                FS = wk.tile([128, 2, nf], bf16, tag="fs", name=f"fs{q0}")
                nc.vector.tensor_mul(FS[:], SF[:], SCc[:, :, 0:ccols])
                csum = wk.tile([128, 2, ip], bf16, tag="cs", name=f"cs{q0}")
                if child_major:
                    fv = FS.rearrange("p t (c j) -> p t c j", c=K)
                    CA = wk.tile([128, 2, ip], bf16, tag="ca", name=f"ca{q0}")
                    nc.gpsimd.tensor_add(CA[:], fv[:, :, 0, :], fv[:, :, 1, :])
                    CB = wk.tile([128, 2, ip], bf16, tag="cb", name=f"cb{q0}")
                    nc.gpsimd.tensor_add(CB[:], fv[:, :, 2, :], fv[:, :, 3, :])
                    nc.vector.tensor_add(csum[:], CA[:], CB[:])
                else:
                    fv = FS.rearrange("p t (j c) -> p t j c", c=2)
                    s2 = wk.tile([128, 2, 2 * ip], bf16, tag="ca",
                                 name=f"ca{q0}")
                    nc.gpsimd.tensor_add(s2[:], fv[:, :, :, 0], fv[:, :, :, 1])
                    s2v = s2.rearrange("p t (j c) -> p t j c", c=2)
                    nc.vector.tensor_add(csum[:], s2v[:, :, :, 0],
                                         s2v[:, :, :, 1])
                # c = ig*ug + csum ; h = og*tanh(c)
                Cd = SCo[:, :, off:off + ip]
                t1 = wk.tile([128, 2, ip], bf16, tag="t1", name=f"t1{q0}")
                nc.vector.tensor_mul(t1[:], Gio[:, 0], GU[:])
                nc.vector.tensor_add(Cd, t1[:], csum[:])
                TC = wk.tile([128, 2, ip], bf16, tag="tc2", name=f"tc2{q0}")
                nc.scalar.activation(TC[:], Cd, Tanh)
                nc.vector.tensor_mul(SHo[:, :, off:off + ip], Gio[:, 1], TC[:])

            # ================= program order =================
            # R0 (L7 leaves) -> RI (internal x-side) -> L6i chunk
            # -> R1 (L6 leaves) -> L5 chunk -> DMA out
            leaf_range(R0[0], R0[1], SH7, SC7, 0)
            p1_resident()
            chunk(0, IPMAX, SH7, SC7, L7P, True, SH6, SC6, 0, last=False)
            leaf_range(R1[0], R1[1], SH6, SC6, 96)
            chunk(IPMAX, NL5, SH6, SC6, NL6, False, SH5, SC5, 0, last=True)

            nc.sync.dma_start(out=out_h_d[:], in_=SH5[:])
            nc.scalar.dma_start(out=out_c_d[:], in_=SC5[:])

    nc.compile()
    return nc


def _get_compiled():
    global _COMPILED
    if _COMPILED is None:
        _COMPILED = _build_device_program()
    return _COMPILED


def _numpy_fallback(xs, child_idx, child_mask, emb_table, Wx, bx, Wh, bh,
                    Wout, bout):
    """Exact sequential scan; only used if the tree isn't the static heap."""
    X = np.asarray(emb_table, dtype=F32)[np.asarray(xs)]
    Wx = np.asarray(Wx, dtype=F32)
    Wh = np.asarray(Wh, dtype=F32)
    bx = np.asarray(bx, dtype=F32)
    bh = np.asarray(bh, dtype=F32)
    gx = np.einsum('ghd,nd->ngh', Wx, X).astype(F32) + bx
    Hb = np.zeros((N, H), dtype=F32)
    Cb = np.zeros((N, H), dtype=F32)
    ci = np.asarray(child_idx)
    cm = np.asarray(child_mask, dtype=F32)
    for i in range(N - 1, -1, -1):
        idx = ci[i]
        m = cm[i][:, None]
        Hc = Hb[idx] * m
        Cc = Cb[idx] * m
        hsum = Hc.sum(0)
        g = gx[i]
        ig = _sigmoid(g[0] + Wh[0] @ hsum + bh[0])
        og = _sigmoid(g[2] + Wh[2] @ hsum + bh[2])
        ug = np.tanh(g[3] + Wh[3] @ hsum + bh[3]).astype(F32)
        f = _sigmoid(g[1] + Hc @ Wh[1].T + bh[1])
        c = ig * ug + (f * Cc).sum(0)
        Hb[i] = og * np.tanh(c).astype(F32)
        Cb[i] = c
    logits = np.asarray(Wout, dtype=F32) @ Hb[0] + np.asarray(bout, dtype=F32)
    return _log_softmax(logits)


def kernel(xs, child_idx, child_mask, emb_table, Wx, bx, Wh, bh, Wout, bout):
    xs = np.asarray(xs)
    if not (np.array_equal(np.asarray(child_idx), _STATIC_IDX)
            and np.array_equal(np.asarray(child_mask, dtype=F32),
                               _STATIC_MASK)):
        return _numpy_fallback(xs, child_idx, child_mask, emb_table, Wx, bx,
                               Wh, bh, Wout, bout)

    from concourse.bass_utils import run_bass_kernel_spmd

    wx, wh, whf, ident = _pack_weights(Wx, bx, Wh, bh)
    xts = _pack_xt(xs, emb_table)
    in_maps = [{"xt": xts[c], "wx": wx, "wh": wh, "whf": whf, "ident": ident}
               for c in range(NCORES)]
    nc = _get_compiled()
    res = run_bass_kernel_spmd(nc, in_maps, core_ids=list(range(NCORES)))

    Hbuf = np.zeros((1365, H), dtype=F32)
    Cbuf = np.zeros((1365, H), dtype=F32)
    for c in range(NCORES):
        _, _, l5 = _PLAN[c]
        oh = np.asarray(res.results[c]["out_h"], dtype=F32)  # [128, 2, 128]
        oc = np.asarray(res.results[c]["out_c"], dtype=F32)
        Hbuf[l5] = np.concatenate([oh[:, 0, :], oh[:, 1, :]], axis=0).T
        Cbuf[l5] = np.concatenate([oc[:, 0, :], oc[:, 1, :]], axis=0).T

    h0 = _host_top(Hbuf, Cbuf, xs, emb_table, Wx, bx, Wh, bh)
    logits = np.asarray(Wout, dtype=F32) @ h0 + np.asarray(bout, dtype=F32)
    return _log_softmax(logits)
